# revision 26
# baseline (speedup 1.0000x reference)
"""Trainium2 Bass kernel: sigmoid multi-head attention (16 heads, S=2048, D=1024,
P=64) + final linear, head-sharded across 8 NeuronCores (2 heads/core).

Reference semantics: concat = attn.reshape(S, -1) is a RAW reshape of the
contiguous [H, S, P] attn array, so output row i draws only from head
h = i // 128:  out[h*128 + r, f] = sum_{u,p} attn[h, 16r+u, p] * W_fin[u*64+p, f]
with u = s%16, r = s//16.  Core c (heads 2c, 2c+1) owns output rows
[256c, 256c+256); the host gather is a concatenation.

v2 design (cost-model driven):
  * The whole kernel runs in a PERMUTED s/t order: position i = u*128 + r
    (u-major), applied on the host to x^T's columns.  Projections, scores,
    sigmoid and attention all inherit the permutation (attention contracts
    over all t, any consistent order is exact).  Benefit: a 128-wide
    permuted-s tile = one u value for all 128 r, so the attention output is
    already grouped by the final linear's contraction chunks.
  * Scores: scoreT [t,128, (h, s,512)] psum per (block, t-tile), fp32r,
    2 matmuls of N=512.  Sigmoid on ScalarE -> sc bf16 SBUF (ScalarE is the
    pipeline bottleneck: 64 x ~1.04us).
  * Attention in NATURAL orientation: out[s,128, p,64] per (tile, head),
    lhsT = sc slice (bf16), rhs = V-natural column half (bf16), N=64 ->
    half the PE cost of the attnT orientation.  All 8 accumulation regions
    of a block share ONE psum bank, pre-seeded by a K=1 zero matmul with
    start=True (sets the whole bank's has_written bits); real matmuls use
    start=False so concurrent per-region accumulation is safe.
  * Final linear: concatT chunks fT_c [k=(u2,p),128, (q:(h,r'))256] bf16 are
    built from the attention output by DMA transpose + 4 small remap DMAs per
    tile, then out[(h,r'),128, f,512] = sum_c fT_c^T @ W_fin[c-chunk] with
    W_fin natural bf16 as the moving operand: 32 matmuls of N=512 total
    (4x fewer PE cycles than the per-u K=64 scheme).  Accumulated in 2
    psum passes (after block 1 and block 3) with DVE adds into SBUF.
  * Emission is software-pipelined: sig(i) | scores(i+2) | attn(i-2) plus
    "slack" projection/final pieces, so stage-P and the final hide under the
    ScalarE-bound sigmoid stream.
"""

import os

os.environ.setdefault("BASS_NEVER_TRACE", "1")

import numpy as np
from contextlib import ExitStack

import jax
import concourse.bacc as bacc
import concourse.bass as bass
import concourse.mybir as mybir
import concourse.tile as tile
from concourse.bass2jax import (
    _bass_exec_p,
    install_neuronx_cc_hook,
    partition_id_tensor,
)
from jax.experimental.shard_map import shard_map
from jax.sharding import Mesh, NamedSharding, PartitionSpec

S, D, H, P, F = 2048, 1024, 16, 64, 1024
NCORES = 8
HL = H // NCORES          # heads per core = 2
P2 = HL * P               # stacked head dim = 128
DCH = D // 128            # 8 contraction chunks
NB = S // 512             # 4 s-blocks (permuted order)
NT = S // 128             # 16 t-tiles
NC_F = 8                  # final-linear contraction chunks of 128

FP32 = mybir.dt.float32
FP32R = mybir.dt.float32r
BF16 = mybir.dt.bfloat16
SIGMOID = mybir.ActivationFunctionType.Sigmoid
ADD = mybir.AluOpType.add


def build_kernel(ctx: ExitStack, tc: tile.TileContext, xt_d, wq_d, wk_d, wv_d,
                 wf_d, id_d, out_d):
    nc = tc.nc

    const_pool = ctx.enter_context(tc.tile_pool(name="const", bufs=1))
    w_pool = ctx.enter_context(tc.tile_pool(name="wts", bufs=1))
    qk_pool = ctx.enter_context(tc.tile_pool(name="qk", bufs=1))
    xt_pool = ctx.enter_context(tc.tile_pool(name="xt", bufs=4))
    sc_pool = ctx.enter_context(tc.tile_pool(name="sc", bufs=6))
    ab_pool = ctx.enter_context(tc.tile_pool(name="ab", bufs=2))
    abt_pool = ctx.enter_context(tc.tile_pool(name="abt", bufs=1))
    ft_pool = ctx.enter_context(tc.tile_pool(name="ft", bufs=1))
    fa_pool = ctx.enter_context(tc.tile_pool(name="fa", bufs=1))
    ot_pool = ctx.enter_context(tc.tile_pool(name="ot", bufs=1))

    # ---- PSUM: pp (P-stage, 2 banks) | ps_s (4 banks) | pa (2 banks) ----
    pp_pool = tc.alloc_tile_pool(name="pp", bufs=1, space="PSUM", side="right")
    ps_s_pool = ctx.enter_context(tc.tile_pool(name="ps_s", bufs=2, space="PSUM"))
    pa_pool = ctx.enter_context(tc.tile_pool(name="pa", bufs=2, space="PSUM"))

    ident = const_pool.tile([128, 128], FP32, tag="ident")
    nc.gpsimd.dma_start(ident, id_d)
    identb = const_pool.tile([128, 128], BF16, tag="identb")
    nc.gpsimd.dma_start(identb, id_d)      # gpsimd DMA casts fp32 -> bf16
    zl = const_pool.tile([1, 128], BF16, tag="zl")
    zr = const_pool.tile([1, 512], BF16, tag="zr")
    nc.vector.memset(zl, 0.0)
    nc.vector.memset(zr, 0.0)
    # preload the sigmoid act table while ScalarE is otherwise idle
    warmt = const_pool.tile([1, 128], BF16, tag="warmt")
    nc.scalar.activation(warmt, zl, SIGMOID)

    # ---------------- input DMAs (three lanes: sync / scalar / gpsimd) ------
    wk = w_pool.tile([128, D], FP32R, tag="wk")
    nc.gpsimd.dma_start(wk, wk_d.bitcast(FP32R))
    wq = w_pool.tile([128, D], FP32R, tag="wq")
    nc.gpsimd.dma_start(wq, wq_d.bitcast(FP32R))
    wv = w_pool.tile([128, D], FP32R, tag="wv")
    nc.gpsimd.dma_start(wv, wv_d.bitcast(FP32R))

    xts = [[None] * DCH for _ in range(NB)]

    def emit_x(sb, ds, engine):
        for d in ds:
            t = xt_pool.tile([128, 512], FP32R, tag=f"x{d}", name=f"xt{sb}_{d}")
            engine.dma_start(
                t, xt_d[d * 128:(d + 1) * 128,
                        sb * 512:(sb + 1) * 512].bitcast(FP32R))
            xts[sb][d] = t

    emit_x(0, range(0, 8), nc.sync)
    emit_x(1, range(0, 8), nc.scalar)   # ScalarE is idle until the stream
    emit_x(2, range(0, 8), nc.sync)
    emit_x(3, range(0, 8), nc.gpsimd)

    wf = w_pool.tile([128, NC_F * F], BF16, tag="wf")
    for c in range(NC_F):
        nc.sync.dma_start(wf[:, c * F:(c + 1) * F],
                          wf_d[:, c * F:(c + 1) * F])

    qt2 = qk_pool.tile([128, S], FP32R, tag="qt2")   # [p2, s-perm]
    kt2 = qk_pool.tile([128, S], FP32R, tag="kt2")   # [p2, t-perm]
    vt2 = qk_pool.tile([128, S], FP32, tag="vt2")    # [p2, t-perm]
    v2n = qk_pool.tile([128, S], BF16, tag="v2n")    # [t_in, j*128 + p2]

    # ---------------- stage-P pieces (emission closures) ----------------
    pj_live = {}

    def piece_proj(w, dst, sb, wname, half=None):
        """Projection piece: half 0 = chunks 0-3 (allocates the pj slot),
        half 1 = chunks 4-7 + psum->SBUF copy, None = whole projection.
        The two halves of one projection must be emitted with no other
        projection's pieces in between (single pj slot)."""
        cols = slice(sb * 512, (sb + 1) * 512)

        def f():
            if half in (0, None):
                pj_live["pj"] = pp_pool.tile([128, 512], FP32, tag="pj",
                                             name=f"pj_{wname}{sb}")
            pj = pj_live["pj"]
            ds = {0: range(0, 4), 1: range(4, 8), None: range(8)}[half]
            for d in ds:
                nc.tensor.matmul(pj, w[:, d * 128:(d + 1) * 128], xts[sb][d],
                                 start=(d == 0), stop=(d == DCH - 1))
            if half in (1, None):
                nc.vector.tensor_copy(dst[:, cols], pj)
        return f

    def piece_vtr(j0, j1):
        """Transpose 128-col tiles [j0, j1) of vt2 into v2n (bf16)."""
        def f():
            for j in range(j0, j1):
                pt = pp_pool.tile([128, 128], FP32, tag="pt", name=f"pt{j}")
                nc.tensor.transpose(pt, vt2[:, j * 128:(j + 1) * 128], ident)
                nc.vector.tensor_copy(v2n[:, j * 128:(j + 1) * 128], pt)
        return f

    # ---------------- stream state ----------------
    pa_tiles = [None] * NB
    sc_tiles = [None] * (NB * NT)
    ps_tiles = [None] * (NB * NT)
    ft_tiles = [None] * NC_F
    fa_tiles = {}
    psf_holder = {}

    # stream order: iters 0..31 interleave blocks 0/1 (b = i%2, j = i//2),
    # then block 2 (j = i-32), then block 3 (j = i-48).  This spreads the
    # t-tile (K/V) deadlines over 32 iters and leaves only block 3's
    # restack + final chunk in the tail.
    def BJ(i):
        if i < 32:
            return i % 2, i // 2
        if i < 48:
            return 2, i - 32
        return 3, i - 48

    def emit_zero(b):
        pa = pa_pool.tile([128, 512], FP32, tag="pa", name=f"pa{b}")
        nc.tensor.matmul(pa, zl, zr, start=True, stop=False,
                         skip_group_check=True)
        pa_tiles[b] = pa

    def emit_scores(i):
        b, j = BJ(i)
        ps = ps_s_pool.tile([128, 1024], FP32, tag="ps_s", name=f"ps{b}_{j}")
        t0, s0 = j * 128, b * 512
        nc.tensor.matmul(ps[:, 0:512], kt2[0:64, t0:t0 + 128],
                         qt2[0:64, s0:s0 + 512])
        nc.tensor.matmul(ps[:, 512:1024], kt2[64:128, t0:t0 + 128],
                         qt2[64:128, s0:s0 + 512])
        ps_tiles[b * NT + j] = ps

    def emit_sig(i):
        b, j = BJ(i)
        sc = sc_pool.tile([128, 1024], BF16, tag="sc", name=f"sc{b}_{j}")
        nc.scalar.activation(sc, ps_tiles[b * NT + j], SIGMOID, scale=1.0 / P)
        sc_tiles[b * NT + j] = sc

    def emit_attn(i):
        b, j = BJ(i)
        pa = pa_tiles[b]
        sc = sc_tiles[b * NT + j]
        t0 = j * 128
        for t4 in range(4):
            for h in range(2):
                nc.tensor.matmul(
                    pa[:, t4 * 128 + h * 64: t4 * 128 + h * 64 + 64],
                    sc[:, h * 512 + t4 * 128: h * 512 + (t4 + 1) * 128],
                    v2n[:, t0 + h * 64: t0 + h * 64 + 64],
                    start=False, stop=(j == NT - 1), skip_group_check=True)

    def emit_abchain(b, tail=False):
        """After attn(b,15): stage, transpose and remap the block's 4 tiles.
        ft layout: [k=(u2,p), (q, h, r')]; each (tau, h) remap is one DMA with
        a 3D dest AP.  In the tail, the idle ScalarE does the staging copy and
        all three DMA-capable engines share the transposes/remaps."""
        ab = ab_pool.tile([128, 512], BF16, tag="ab", name=f"ab{b}")
        if tail:
            nc.scalar.activation(ab, pa_tiles[b],
                                 mybir.ActivationFunctionType.Copy)
        else:
            nc.vector.tensor_copy(ab, pa_tiles[b])
        lanes = ((nc.gpsimd, nc.scalar, nc.sync) if tail
                 else (nc.gpsimd, nc.sync))
        li = 0
        for t4 in range(4):
            tau = 4 * b + t4
            c, u2 = tau // 2, tau % 2
            abt = abt_pool.tile([128, 128], BF16, tag=f"abt{tau}",
                                name=f"abt{tau}")
            nc.sync.dma_start_transpose(abt, ab[:, t4 * 128:(t4 + 1) * 128])
            if u2 == 0:
                ft_tiles[c] = ft_pool.tile([128, 256], BF16, tag=f"ft{c}",
                                           name=f"ft{c}")
            ft = ft_tiles[c]
            for h in range(2):
                for qq in range(2):
                    eng = lanes[li % len(lanes)]
                    li += 1
                    # ft cols are (q, h, r'); abt cols are r = 64q + r'.
                    # Plain 2D slices only: exotic APs break Tile's
                    # write-dependency tracking (observed race).
                    eng.dma_start(
                        ft[u2 * 64:(u2 + 1) * 64,
                           qq * 128 + h * 64: qq * 128 + h * 64 + 64],
                        abt[h * 64:(h + 1) * 64, qq * 64:(qq + 1) * 64])

    def final_pieces(phase):
        """phase 0: chunks 0-3 -> fa copy; 1: chunks 4-5 -> fa add;
        2: chunks 6-7 -> ot = fa + psf, DMA out."""
        pieces = []
        state = {}
        cs = ((0, 1, 2, 3), (4, 5), (6, 7))[phase]

        def mk_mm(q, fc, sub):
            def f():
                psf_pool = psf_holder["pool"]
                if sub[0] == cs[0]:
                    state[(q, fc)] = psf_pool.tile(
                        [128, 512], FP32, tag="psf", name=f"psf{phase}{q}{fc}")
                psf = state[(q, fc)]
                if phase == 2 and sub[0] == cs[0]:
                    # zero-prefix: holds the PE p-state through the restack
                    # DMA window and pre-seeds the accumulation group
                    for z in range(3):
                        nc.tensor.matmul(psf, zl, zr, start=(z == 0),
                                         stop=False, skip_group_check=True)
                for c in sub:
                    lhsT = ft_tiles[c][:, q * 128:(q + 1) * 128]
                    nc.tensor.matmul(
                        psf, lhsT,
                        wf[:, c * F + fc * 512: c * F + fc * 512 + 512],
                        start=(c == cs[0] and phase != 2),
                        stop=(c == cs[-1] and phase != 2),
                        skip_group_check=(phase == 2))
                if phase == 2 and sub[-1] == cs[-1]:
                    # merge the pass-0/1 accumulator: psf += I.T @ fa
                    nc.tensor.matmul(psf, identb, fa_tiles[(q, fc)],
                                     start=False, stop=True,
                                     skip_group_check=True)
            return f

        def mk_tail(q, fc):
            def f():
                psf = state[(q, fc)]
                if phase == 0:
                    fa = fa_pool.tile([128, 512], BF16, tag=f"fa{q}{fc}",
                                      name=f"fa{q}{fc}")
                    nc.vector.tensor_copy(fa, psf)
                    fa_tiles[(q, fc)] = fa
                elif phase == 1:
                    nc.vector.tensor_tensor(fa_tiles[(q, fc)],
                                            fa_tiles[(q, fc)], psf, ADD)
                else:
                    # psf already holds fa (identity merge); extract on the
                    # two idle lanes in parallel
                    ot = ot_pool.tile([128, 512], BF16, tag=f"ot{q}{fc}",
                                      name=f"ot{q}{fc}")
                    if (q + fc) % 2 == 0:
                        nc.scalar.activation(
                            ot, psf, mybir.ActivationFunctionType.Copy)
                    else:
                        nc.vector.tensor_copy(ot, psf)
                    eng = nc.sync if (q + fc) % 2 == 0 else nc.scalar
                    eng.dma_start(
                        out_d[:, (q * 2 + fc) * 512:(q * 2 + fc + 1) * 512],
                        ot)
            return f

        for q in range(2):
            for fc in range(2):
                if phase == 0:
                    pieces.append(mk_mm(q, fc, (0, 1)))
                    pieces.append(mk_mm(q, fc, (2, 3)))
                else:
                    pieces.append(mk_mm(q, fc, cs))
                pieces.append(mk_tail(q, fc))
        return pieces

    # ---------------- prologue ----------------
    # Emission order IS dependency order: never emit a consumer before its
    # producer (the scheduler honors emission order per engine).
    piece_proj(wk, kt2, 0, "k")()
    piece_proj(wq, qt2, 0, "q")()
    emit_zero(0)
    emit_zero(1)
    emit_scores(0)
    piece_proj(wq, qt2, 1, "q")()       # Q(sb1) for the interleaved stream
    emit_scores(1)
    piece_proj(wv, vt2, 0, "v")()
    piece_vtr(0, 2)()
    piece_vtr(2, 4)()

    # ---------------- slack schedule ----------------
    sched = {i: [] for i in range(NB * NT + 8)}

    def put_proj(i0, w, dst, sb, wname):
        sched[i0].append(piece_proj(w, dst, sb, wname, 0))
        sched[i0 + 1].append(piece_proj(w, dst, sb, wname, 1))

    put_proj(0, wk, kt2, 1, "k")
    put_proj(2, wv, vt2, 1, "v")
    sched[4].append(piece_vtr(4, 6))
    sched[5].append(piece_vtr(6, 8))
    put_proj(8, wk, kt2, 2, "k")
    put_proj(10, wv, vt2, 2, "v")
    sched[12].append(piece_vtr(8, 10))
    sched[13].append(piece_vtr(10, 12))
    put_proj(16, wk, kt2, 3, "k")
    put_proj(18, wv, vt2, 3, "v")
    sched[20].append(piece_vtr(12, 14))
    sched[21].append(piece_vtr(14, 16))
    put_proj(26, wq, qt2, 2, "q")
    put_proj(30, wq, qt2, 3, "q")
    sched[33].append(lambda: emit_zero(2))
    sched[48].append(lambda: emit_zero(3))

    # ---------------- main stream ----------------
    for i in range(NB * NT):
        emit_sig(i)
        if i + 2 < NB * NT:
            emit_scores(i + 2)
        if i - 2 >= 0:
            emit_attn(i - 2)
            bb, jj = BJ(i - 2)
            if jj == NT - 1:
                emit_abchain(bb)
                if bb == 1:
                    # all stage-P work is emitted; free its PSUM for psf
                    pp_pool.release()
                    cm = tc.tile_pool(name="psf", bufs=2, space="PSUM")
                    psf_holder["cm"] = cm
                    psf_holder["pool"] = cm.__enter__()
                    for k, p in enumerate(final_pieces(0)):
                        sched[36 + (k * 10) // 12].append(p)
                elif bb == 2:
                    for k, p in enumerate(final_pieces(1)):
                        sched[52 + (k * 8) // 8].append(p)
        for p in sched[i]:
            p()

    # ---------------- tail ----------------
    emit_attn(NB * NT - 2)
    emit_attn(NB * NT - 1)
    emit_abchain(NB - 1, tail=True)
    for p in final_pieces(2):
        p()
    psf_holder["cm"].__exit__(None, None, None)


def build_bass(replicas: int = 1) -> bass.Bass:
    nc = bacc.Bacc("TRN2", target_bir_lowering=False, debug=False,
                   num_devices=NCORES)
    xt_d = nc.dram_tensor("xt", [D, S], FP32, kind="ExternalInput").ap()
    wq_d = nc.dram_tensor("wq", [128, D], FP32, kind="ExternalInput").ap()
    wk_d = nc.dram_tensor("wk", [128, D], FP32, kind="ExternalInput").ap()
    wv_d = nc.dram_tensor("wv", [128, D], FP32, kind="ExternalInput").ap()
    wf_d = nc.dram_tensor("wf", [128, NC_F * F], BF16, kind="ExternalInput").ap()
    id_d = nc.dram_tensor("ident", [128, 128], FP32, kind="ExternalInput").ap()
    # compact layout: [ (h,r'), (q,fc)*512 + f' ] — host unscatters rows
    out_d = nc.dram_tensor("out", [128, 4 * 512], BF16, kind="ExternalOutput").ap()
    with tile.TileContext(nc) as tc:
        for _ in range(replicas):
            with ExitStack() as ctx:
                build_kernel(ctx, tc, xt_d, wq_d, wk_d, wv_d, wf_d, id_d,
                             out_d)
    nc.finalize()
    return nc


_NC_CACHE = None
_EXEC_CACHE = None
LAST_DEV_ARGS = None
LAST_OUT_NAMES = None


def _get_nc():
    global _NC_CACHE
    if _NC_CACHE is None:
        _NC_CACHE = build_bass()
    return _NC_CACHE


def _get_executor():
    """Compile the SPMD PJRT executable once (mirrors bass2jax.run_bass_via_pjrt,
    minus output-buffer donation)."""
    global _EXEC_CACHE
    if _EXEC_CACHE is not None:
        return _EXEC_CACHE
    import concourse.mybir as mybir

    nc = _get_nc()
    install_neuronx_cc_hook()
    partition_name = (nc.partition_id_tensor.name
                      if nc.partition_id_tensor else None)
    in_names, out_names, out_avals = [], [], []
    out_shapes = []
    for alloc in nc.m.functions[0].allocations:
        if not isinstance(alloc, mybir.MemoryLocationSet):
            continue
        name = alloc.memorylocations[0].name
        if alloc.kind == "ExternalInput":
            if name != partition_name:
                in_names.append(name)
        elif alloc.kind == "ExternalOutput":
            shape = tuple(alloc.tensor_shape)
            dtype = mybir.dt.np(alloc.dtype)
            out_names.append(name)
            out_avals.append(jax.core.ShapedArray(shape, dtype))
            out_shapes.append((shape, dtype))
    n_params = len(in_names)
    all_names = list(in_names) + list(out_names)
    if partition_name is not None:
        all_names.append(partition_name)

    def _body(*args):
        operands = list(args)
        if partition_name is not None:
            operands.append(partition_id_tensor())
        outs = _bass_exec_p.bind(
            *operands,
            out_avals=tuple(out_avals),
            in_names=tuple(all_names),
            out_names=tuple(out_names),
            lowering_input_output_aliases=(),
            sim_require_finite=True,
            sim_require_nnan=True,
            nc=nc,
        )
        return tuple(outs)

    devices = jax.devices()[:NCORES]
    mesh = Mesh(np.asarray(devices), ("core",))
    n_args = n_params + len(out_names)
    sharded = jax.jit(shard_map(
        _body, mesh=mesh,
        in_specs=(PartitionSpec("core"),) * n_args,
        out_specs=(PartitionSpec("core"),) * len(out_names),
        check_rep=False))
    _EXEC_CACHE = (sharded, mesh, in_names, out_names, out_shapes)
    return _EXEC_CACHE


def _run_spmd(in_maps):
    """Execute on all cores; returns list of per-core {name: np.ndarray}."""
    global LAST_DEV_ARGS, LAST_OUT_NAMES
    sharded, mesh, in_names, out_names, out_shapes = _get_executor()
    sh = NamedSharding(mesh, PartitionSpec("core"))
    args = [np.concatenate([im[name] for im in in_maps], axis=0)
            for name in in_names]
    for shape, dtype in out_shapes:
        args.append(np.zeros((NCORES * shape[0],) + shape[1:], dtype))
    dev_args = [jax.device_put(a, sh) for a in args]
    LAST_DEV_ARGS = dev_args
    LAST_OUT_NAMES = out_names
    outs = sharded(*dev_args)
    jax.block_until_ready(outs)
    results = []
    for c in range(NCORES):
        res = {}
        for i, name in enumerate(out_names):
            g = np.asarray(outs[i])
            d0 = g.shape[0] // NCORES
            res[name] = g[c * d0:(c + 1) * d0]
        results.append(res)
    return results


def _layout_w(w, c):
    """[H, D, P] global weights -> per-core [128, D] stationary layout:
    out[di, dc*128 + (h*64+p)] = w[2c+h, dc*128+di, p]"""
    wl = np.transpose(w[HL * c:HL * (c + 1)], (1, 0, 2)).reshape(D, P2)
    wl = wl.reshape(DCH, 128, P2).transpose(1, 0, 2).reshape(128, DCH * P2)
    return np.ascontiguousarray(wl, dtype=np.float32)


def make_in_maps(x, Qw, Kw, Vw, W_fin):
    import ml_dtypes
    x = np.asarray(x, dtype=np.float32)
    Qw = np.asarray(Qw, dtype=np.float32)
    Kw = np.asarray(Kw, dtype=np.float32)
    Vw = np.asarray(Vw, dtype=np.float32)
    W_fin = np.asarray(W_fin, dtype=np.float32)

    # u-major column permutation: position i = u*128 + r  <->  s = 16*r + u
    xt = np.ascontiguousarray(x.T)                      # [D, S]
    xtp = np.ascontiguousarray(
        xt.reshape(D, 128, 16).transpose(0, 2, 1).reshape(D, S))
    ident = np.eye(128, dtype=np.float32)
    # wf: natural W_fin contraction chunks [128, c*F + f], bf16
    wf = np.ascontiguousarray(
        W_fin.reshape(NC_F, 128, F).transpose(1, 0, 2).reshape(128, NC_F * F)
    ).astype(ml_dtypes.bfloat16)

    in_maps = []
    for c in range(NCORES):
        in_maps.append({
            "xt": xtp,
            "wq": _layout_w(Qw, c),
            "wk": _layout_w(Kw, c),
            "wv": _layout_w(Vw, c),
            "wf": wf,
            "ident": ident,
        })
    return in_maps


def assemble_out(results, b_fin):
    b_fin = np.asarray(b_fin, dtype=np.float32)
    cores = []
    for c in range(NCORES):
        buf = results[c]["out"].astype(np.float32)      # [128, 2048]
        v = buf.reshape(2, 64, 2, 2, 512)               # [h, r', q, fc, f]
        cores.append(v.transpose(0, 2, 1, 3, 4).reshape(256, F))
    out = np.concatenate(cores, axis=0)
    return (out + b_fin).astype(np.float32)


def kernel(x, Qw, Kw, Vw, W_fin, b_fin):
    in_maps = make_in_maps(x, Qw, Kw, Vw, W_fin)
    results = _run_spmd(in_maps)
    return assemble_out(results, b_fin)


# revision 27
# speedup vs baseline: 1.0559x; 1.0559x over previous
"""Trainium2 Bass kernel: sigmoid multi-head attention (16 heads, S=2048, D=1024,
P=64) + final linear, head-sharded across 8 NeuronCores (2 heads/core).

Reference semantics: concat = attn.reshape(S, -1) is a RAW reshape of the
contiguous [H, S, P] attn array, so output row i draws only from head
h = i // 128:  out[h*128 + r, f] = sum_{u,p} attn[h, 16r+u, p] * W_fin[u*64+p, f]
with u = s%16, r = s//16.  Core c (heads 2c, 2c+1) owns output rows
[256c, 256c+256); the host gather is a concatenation.

v2 design (cost-model driven):
  * The whole kernel runs in a PERMUTED s/t order: position i = u*128 + r
    (u-major), applied on the host to x^T's columns.  Projections, scores,
    sigmoid and attention all inherit the permutation (attention contracts
    over all t, any consistent order is exact).  Benefit: a 128-wide
    permuted-s tile = one u value for all 128 r, so the attention output is
    already grouped by the final linear's contraction chunks.
  * Scores: scoreT [t,128, (h, s,512)] psum per (block, t-tile), fp32r,
    2 matmuls of N=512.  Sigmoid on ScalarE -> sc bf16 SBUF (ScalarE is the
    pipeline bottleneck: 64 x ~1.04us).
  * Attention in NATURAL orientation: out[s,128, p,64] per (tile, head),
    lhsT = sc slice (bf16), rhs = V-natural column half (bf16), N=64 ->
    half the PE cost of the attnT orientation.  All 8 accumulation regions
    of a block share ONE psum bank, pre-seeded by a K=1 zero matmul with
    start=True (sets the whole bank's has_written bits); real matmuls use
    start=False so concurrent per-region accumulation is safe.
  * Final linear: concatT chunks fT_c [k=(u2,p),128, (q:(h,r'))256] bf16 are
    built from the attention output by DMA transpose + 4 small remap DMAs per
    tile, then out[(h,r'),128, f,512] = sum_c fT_c^T @ W_fin[c-chunk] with
    W_fin natural bf16 as the moving operand: 32 matmuls of N=512 total
    (4x fewer PE cycles than the per-u K=64 scheme).  Accumulated in 2
    psum passes (after block 1 and block 3) with DVE adds into SBUF.
  * Emission is software-pipelined: sig(i) | scores(i+2) | attn(i-2) plus
    "slack" projection/final pieces, so stage-P and the final hide under the
    ScalarE-bound sigmoid stream.
"""

import os

os.environ.setdefault("BASS_NEVER_TRACE", "1")

import numpy as np
from contextlib import ExitStack

import jax
import concourse.bacc as bacc
import concourse.bass as bass
import concourse.mybir as mybir
import concourse.tile as tile
from concourse.bass2jax import (
    _bass_exec_p,
    install_neuronx_cc_hook,
    partition_id_tensor,
)
from jax.experimental.shard_map import shard_map
from jax.sharding import Mesh, NamedSharding, PartitionSpec

S, D, H, P, F = 2048, 1024, 16, 64, 1024
NCORES = 8
HL = H // NCORES          # heads per core = 2
P2 = HL * P               # stacked head dim = 128
DCH = D // 128            # 8 contraction chunks
NB = S // 512             # 4 s-blocks (permuted order)
NT = S // 128             # 16 t-tiles
NC_F = 8                  # final-linear contraction chunks of 128

FP32 = mybir.dt.float32
FP32R = mybir.dt.float32r
BF16 = mybir.dt.bfloat16
SIGMOID = mybir.ActivationFunctionType.Sigmoid
ADD = mybir.AluOpType.add


def build_kernel(ctx: ExitStack, tc: tile.TileContext, xt_d, wq_d, wk_d, wv_d,
                 wf_d, id_d, out_d):
    nc = tc.nc

    const_pool = ctx.enter_context(tc.tile_pool(name="const", bufs=1))
    w_pool = ctx.enter_context(tc.tile_pool(name="wts", bufs=1))
    qk_pool = ctx.enter_context(tc.tile_pool(name="qk", bufs=1))
    xt_pool = ctx.enter_context(tc.tile_pool(name="xt", bufs=4))
    sc_pool = ctx.enter_context(tc.tile_pool(name="sc", bufs=6))
    ab_pool = ctx.enter_context(tc.tile_pool(name="ab", bufs=2))
    abt_pool = ctx.enter_context(tc.tile_pool(name="abt", bufs=1))
    ft_pool = ctx.enter_context(tc.tile_pool(name="ft", bufs=1))
    fa_pool = ctx.enter_context(tc.tile_pool(name="fa", bufs=1))
    ot_pool = ctx.enter_context(tc.tile_pool(name="ot", bufs=1))

    # ---- PSUM: pp (P-stage, 2 banks) | ps_s (4 banks) | pa (2 banks) ----
    pp_pool = tc.alloc_tile_pool(name="pp", bufs=1, space="PSUM", side="right")
    ps_s_pool = ctx.enter_context(tc.tile_pool(name="ps_s", bufs=2, space="PSUM"))
    pa_pool = ctx.enter_context(tc.tile_pool(name="pa", bufs=2, space="PSUM"))

    ident = const_pool.tile([128, 128], FP32, tag="ident")
    nc.gpsimd.dma_start(ident, id_d)
    identb = const_pool.tile([128, 128], BF16, tag="identb")
    nc.gpsimd.dma_start(identb, id_d)      # gpsimd DMA casts fp32 -> bf16
    zl = const_pool.tile([1, 128], BF16, tag="zl")
    zr = const_pool.tile([1, 512], BF16, tag="zr")
    nc.vector.memset(zl, 0.0)
    nc.vector.memset(zr, 0.0)

    # ---------------- input DMAs (three lanes: sync / scalar / gpsimd) ------
    # gpsimd queue: ident, wk, wq, wv, then x shares (weights gate stage-P)
    wk = w_pool.tile([128, D], FP32R, tag="wk")
    nc.gpsimd.dma_start(wk, wk_d.bitcast(FP32R))
    wq = w_pool.tile([128, D], FP32R, tag="wq")
    nc.gpsimd.dma_start(wq, wq_d.bitcast(FP32R))
    wv = w_pool.tile([128, D], FP32R, tag="wv")
    nc.gpsimd.dma_start(wv, wv_d.bitcast(FP32R))

    xts = [[None] * DCH for _ in range(NB)]

    def emit_x(sb, ds, engine):
        for d in ds:
            t = xt_pool.tile([128, 512], FP32R, tag=f"x{d}", name=f"xt{sb}_{d}")
            engine.dma_start(
                t, xt_d[d * 128:(d + 1) * 128,
                        sb * 512:(sb + 1) * 512].bitcast(FP32R))
            xts[sb][d] = t

    # x0/x1 land first (critical path to the first sigmoids); the ScalarE
    # lane must finish all its DMAs before the first sigmoid is enqueued.
    emit_x(0, range(0, 4), nc.sync)
    emit_x(0, range(4, 8), nc.scalar)
    # preload the sigmoid act table while ScalarE is otherwise idle
    warmt = const_pool.tile([1, 128], BF16, tag="warmt")
    nc.scalar.activation(warmt, zl, SIGMOID)
    emit_x(1, range(0, 4), nc.sync)
    emit_x(1, range(4, 8), nc.gpsimd)
    emit_x(2, range(0, 8), nc.sync)
    emit_x(3, range(0, 8), nc.gpsimd)

    wf = w_pool.tile([128, NC_F * F], BF16, tag="wf")
    for c in range(NC_F):
        nc.sync.dma_start(wf[:, c * F:(c + 1) * F],
                          wf_d[:, c * F:(c + 1) * F])

    qt2 = qk_pool.tile([128, S], FP32R, tag="qt2")   # [p2, s-perm]
    kt2 = qk_pool.tile([128, S], FP32R, tag="kt2")   # [p2, t-perm]
    vt2 = qk_pool.tile([128, S], FP32, tag="vt2")    # [p2, t-perm]
    v2n = qk_pool.tile([128, S], BF16, tag="v2n")    # [t_in, j*128 + p2]

    # ---------------- stage-P pieces (emission closures) ----------------
    pj_live = {}

    def piece_proj(w, dst, sb, wname, half=None):
        """Projection piece: half 0 = chunks 0-3 (allocates the pj slot),
        half 1 = chunks 4-7 + psum->SBUF copy, None = whole projection.
        The two halves of one projection must be emitted with no other
        projection's pieces in between (single pj slot)."""
        cols = slice(sb * 512, (sb + 1) * 512)

        def f():
            if half in (0, None):
                pj_live["pj"] = pp_pool.tile([128, 512], FP32, tag="pj",
                                             name=f"pj_{wname}{sb}")
            pj = pj_live["pj"]
            ds = {0: range(0, 4), 1: range(4, 8), None: range(8)}[half]
            for d in ds:
                nc.tensor.matmul(pj, w[:, d * 128:(d + 1) * 128], xts[sb][d],
                                 start=(d == 0), stop=(d == DCH - 1))
            if half in (1, None):
                nc.vector.tensor_copy(dst[:, cols], pj)
        return f

    def piece_vtr(j0, j1):
        """Transpose 128-col tiles [j0, j1) of vt2 into v2n (bf16)."""
        def f():
            for j in range(j0, j1):
                pt = pp_pool.tile([128, 128], FP32, tag="pt", name=f"pt{j}")
                nc.tensor.transpose(pt, vt2[:, j * 128:(j + 1) * 128], ident)
                nc.vector.tensor_copy(v2n[:, j * 128:(j + 1) * 128], pt)
        return f

    # ---------------- stream state ----------------
    pa_tiles = [None] * NB
    sc_tiles = [None] * (NB * NT)
    ps_tiles = [None] * (NB * NT)
    ft_tiles = [None] * NC_F
    fa_tiles = {}
    psf_holder = {}

    # stream order: iters 0..31 interleave blocks 0/1 (b = i%2, j = i//2),
    # then block 2 (j = i-32), then block 3 (j = i-48).  This spreads the
    # t-tile (K/V) deadlines over 32 iters and leaves only block 3's
    # restack + final chunk in the tail.
    def BJ(i):
        if i < 32:
            return i % 2, i // 2
        if i < 48:
            return 2, i - 32
        return 3, i - 48

    def emit_zero(b):
        pa = pa_pool.tile([128, 512], FP32, tag="pa", name=f"pa{b}")
        nc.tensor.matmul(pa, zl, zr, start=True, stop=False,
                         skip_group_check=True)
        pa_tiles[b] = pa

    def emit_scores(i):
        b, j = BJ(i)
        ps = ps_s_pool.tile([128, 1024], FP32, tag="ps_s", name=f"ps{b}_{j}")
        t0, s0 = j * 128, b * 512
        nc.tensor.matmul(ps[:, 0:512], kt2[0:64, t0:t0 + 128],
                         qt2[0:64, s0:s0 + 512])
        nc.tensor.matmul(ps[:, 512:1024], kt2[64:128, t0:t0 + 128],
                         qt2[64:128, s0:s0 + 512])
        ps_tiles[b * NT + j] = ps

    def emit_sig(i):
        b, j = BJ(i)
        sc = sc_pool.tile([128, 1024], BF16, tag="sc", name=f"sc{b}_{j}")
        nc.scalar.activation(sc, ps_tiles[b * NT + j], SIGMOID, scale=1.0 / P)
        sc_tiles[b * NT + j] = sc

    def emit_attn(i):
        b, j = BJ(i)
        pa = pa_tiles[b]
        sc = sc_tiles[b * NT + j]
        t0 = j * 128
        for t4 in range(4):
            for h in range(2):
                nc.tensor.matmul(
                    pa[:, t4 * 128 + h * 64: t4 * 128 + h * 64 + 64],
                    sc[:, h * 512 + t4 * 128: h * 512 + (t4 + 1) * 128],
                    v2n[:, t0 + h * 64: t0 + h * 64 + 64],
                    start=False, stop=(j == NT - 1), skip_group_check=True)

    def emit_abchain(b, tail=False):
        """After attn(b,15): stage, transpose and remap the block's 4 tiles.
        ft layout: [k=(u2,p), (q, h, r')]; each (tau, h) remap is one DMA with
        a 3D dest AP.  In the tail, the idle ScalarE does the staging copy and
        all three DMA-capable engines share the transposes/remaps."""
        ab = ab_pool.tile([128, 512], BF16, tag="ab", name=f"ab{b}")
        if tail:
            nc.scalar.activation(ab, pa_tiles[b],
                                 mybir.ActivationFunctionType.Copy)
        else:
            nc.vector.tensor_copy(ab, pa_tiles[b])
        lanes = ((nc.scalar, nc.sync) if tail
                 else (nc.gpsimd, nc.sync))
        li = 0
        for t4 in range(4):
            tau = 4 * b + t4
            c, u2 = tau // 2, tau % 2
            abt = abt_pool.tile([128, 128], BF16, tag=f"abt{tau}",
                                name=f"abt{tau}")
            nc.sync.dma_start_transpose(abt, ab[:, t4 * 128:(t4 + 1) * 128])
            if u2 == 0:
                ft_tiles[c] = ft_pool.tile([128, 256], BF16, tag=f"ft{c}",
                                           name=f"ft{c}")
            ft = ft_tiles[c]
            for h in range(2):
                eng = lanes[li % len(lanes)]
                li += 1
                # ft cols are (q, h, r'); abt cols are r = 64q + r'.
                # rearrange + plain slice only — int-indexed APs break
                # Tile's write-dependency tracking (observed race).
                dst = ft[u2 * 64:(u2 + 1) * 64, :].rearrange(
                    "p (q x) -> p q x", q=2, x=128)[:, :, h * 64:(h + 1) * 64]
                sr = abt[h * 64:(h + 1) * 64, :].rearrange(
                    "p (q r) -> p q r", q=2, r=64)
                eng.dma_start(dst, sr)

    def final_pieces(phase):
        """phase 0: chunks 0-3 -> fa copy; 1: chunks 4-5 -> fa add;
        2: chunks 6-7 -> ot = fa + psf, DMA out."""
        pieces = []
        state = {}
        cs = ((0, 1, 2, 3), (4, 5), (6, 7))[phase]

        def mk_mm(q, fc, sub):
            def f():
                psf_pool = psf_holder["pool"]
                if sub[0] == cs[0]:
                    state[(q, fc)] = psf_pool.tile(
                        [128, 512], FP32, tag="psf", name=f"psf{phase}{q}{fc}")
                psf = state[(q, fc)]
                for c in sub:
                    lhsT = ft_tiles[c][:, q * 128:(q + 1) * 128]
                    nc.tensor.matmul(
                        psf, lhsT,
                        wf[:, c * F + fc * 512: c * F + fc * 512 + 512],
                        start=(c == cs[0]), stop=(c == cs[-1]))
            return f

        def mk_tail(q, fc):
            def f():
                psf = state[(q, fc)]
                if phase == 0:
                    fa = fa_pool.tile([128, 512], FP32, tag=f"fa{q}{fc}",
                                      name=f"fa{q}{fc}")
                    nc.vector.tensor_copy(fa, psf)
                    fa_tiles[(q, fc)] = fa
                elif phase == 1:
                    nc.vector.tensor_tensor(fa_tiles[(q, fc)],
                                            fa_tiles[(q, fc)], psf, ADD)
                else:
                    ot = ot_pool.tile([128, 512], BF16, tag=f"ot{q}{fc}",
                                      name=f"ot{q}{fc}")
                    nc.vector.tensor_tensor(ot, fa_tiles[(q, fc)], psf, ADD)
                    eng = nc.sync if (q + fc) % 2 == 0 else nc.scalar
                    eng.dma_start(
                        out_d[:, (q * 2 + fc) * 512:(q * 2 + fc + 1) * 512],
                        ot)
            return f

        for q in range(2):
            for fc in range(2):
                if phase == 0:
                    pieces.append(mk_mm(q, fc, (0, 1)))
                    pieces.append(mk_mm(q, fc, (2, 3)))
                else:
                    pieces.append(mk_mm(q, fc, cs))
                pieces.append(mk_tail(q, fc))
        return pieces

    # ---------------- prologue ----------------
    # Emission order IS dependency order: never emit a consumer before its
    # producer (the scheduler honors emission order per engine).
    piece_proj(wk, kt2, 0, "k")()
    piece_proj(wq, qt2, 0, "q")()
    emit_zero(0)
    emit_zero(1)
    emit_scores(0)
    piece_proj(wq, qt2, 1, "q")()       # Q(sb1) for the interleaved stream
    emit_scores(1)
    piece_proj(wv, vt2, 0, "v")()
    piece_vtr(0, 2)()
    piece_vtr(2, 4)()

    # ---------------- slack schedule ----------------
    sched = {i: [] for i in range(NB * NT + 8)}

    def put_proj(i0, w, dst, sb, wname):
        sched[i0].append(piece_proj(w, dst, sb, wname, 0))
        sched[i0 + 1].append(piece_proj(w, dst, sb, wname, 1))

    put_proj(0, wk, kt2, 1, "k")
    put_proj(2, wv, vt2, 1, "v")
    sched[4].append(piece_vtr(4, 6))
    sched[5].append(piece_vtr(6, 8))
    put_proj(8, wk, kt2, 2, "k")
    put_proj(10, wv, vt2, 2, "v")
    sched[12].append(piece_vtr(8, 10))
    sched[13].append(piece_vtr(10, 12))
    put_proj(16, wk, kt2, 3, "k")
    put_proj(18, wv, vt2, 3, "v")
    sched[20].append(piece_vtr(12, 14))
    sched[21].append(piece_vtr(14, 16))
    put_proj(26, wq, qt2, 2, "q")
    put_proj(30, wq, qt2, 3, "q")
    sched[33].append(lambda: emit_zero(2))
    sched[48].append(lambda: emit_zero(3))

    # ---------------- main stream ----------------
    for i in range(NB * NT):
        emit_sig(i)
        if i + 2 < NB * NT:
            emit_scores(i + 2)
        if i - 2 >= 0:
            emit_attn(i - 2)
            bb, jj = BJ(i - 2)
            if jj == NT - 1:
                emit_abchain(bb)
                if bb == 1:
                    # all stage-P work is emitted; free its PSUM for psf
                    pp_pool.release()
                    cm = tc.tile_pool(name="psf", bufs=2, space="PSUM")
                    psf_holder["cm"] = cm
                    psf_holder["pool"] = cm.__enter__()
                    for k, p in enumerate(final_pieces(0)):
                        sched[36 + (k * 10) // 12].append(p)
                elif bb == 2:
                    for k, p in enumerate(final_pieces(1)):
                        sched[52 + (k * 8) // 8].append(p)
        for p in sched[i]:
            p()

    # ---------------- tail ----------------
    emit_attn(NB * NT - 2)
    emit_attn(NB * NT - 1)
    emit_abchain(NB - 1, tail=True)
    for p in final_pieces(2):
        p()
    psf_holder["cm"].__exit__(None, None, None)


def build_bass(replicas: int = 1) -> bass.Bass:
    nc = bacc.Bacc("TRN2", target_bir_lowering=False, debug=False,
                   num_devices=NCORES)
    xt_d = nc.dram_tensor("xt", [D, S], FP32, kind="ExternalInput").ap()
    wq_d = nc.dram_tensor("wq", [128, D], FP32, kind="ExternalInput").ap()
    wk_d = nc.dram_tensor("wk", [128, D], FP32, kind="ExternalInput").ap()
    wv_d = nc.dram_tensor("wv", [128, D], FP32, kind="ExternalInput").ap()
    wf_d = nc.dram_tensor("wf", [128, NC_F * F], BF16, kind="ExternalInput").ap()
    id_d = nc.dram_tensor("ident", [128, 128], FP32, kind="ExternalInput").ap()
    # compact layout: [ (h,r'), (q,fc)*512 + f' ] — host unscatters rows
    out_d = nc.dram_tensor("out", [128, 4 * 512], BF16, kind="ExternalOutput").ap()
    with tile.TileContext(nc) as tc:
        for _ in range(replicas):
            with ExitStack() as ctx:
                build_kernel(ctx, tc, xt_d, wq_d, wk_d, wv_d, wf_d, id_d,
                             out_d)
    nc.finalize()
    return nc


_NC_CACHE = None
_EXEC_CACHE = None
LAST_DEV_ARGS = None
LAST_OUT_NAMES = None


def _get_nc():
    global _NC_CACHE
    if _NC_CACHE is None:
        _NC_CACHE = build_bass()
    return _NC_CACHE


def _get_executor():
    """Compile the SPMD PJRT executable once (mirrors bass2jax.run_bass_via_pjrt,
    minus output-buffer donation)."""
    global _EXEC_CACHE
    if _EXEC_CACHE is not None:
        return _EXEC_CACHE
    import concourse.mybir as mybir

    nc = _get_nc()
    install_neuronx_cc_hook()
    partition_name = (nc.partition_id_tensor.name
                      if nc.partition_id_tensor else None)
    in_names, out_names, out_avals = [], [], []
    out_shapes = []
    for alloc in nc.m.functions[0].allocations:
        if not isinstance(alloc, mybir.MemoryLocationSet):
            continue
        name = alloc.memorylocations[0].name
        if alloc.kind == "ExternalInput":
            if name != partition_name:
                in_names.append(name)
        elif alloc.kind == "ExternalOutput":
            shape = tuple(alloc.tensor_shape)
            dtype = mybir.dt.np(alloc.dtype)
            out_names.append(name)
            out_avals.append(jax.core.ShapedArray(shape, dtype))
            out_shapes.append((shape, dtype))
    n_params = len(in_names)
    all_names = list(in_names) + list(out_names)
    if partition_name is not None:
        all_names.append(partition_name)

    def _body(*args):
        operands = list(args)
        if partition_name is not None:
            operands.append(partition_id_tensor())
        outs = _bass_exec_p.bind(
            *operands,
            out_avals=tuple(out_avals),
            in_names=tuple(all_names),
            out_names=tuple(out_names),
            lowering_input_output_aliases=(),
            sim_require_finite=True,
            sim_require_nnan=True,
            nc=nc,
        )
        return tuple(outs)

    devices = jax.devices()[:NCORES]
    mesh = Mesh(np.asarray(devices), ("core",))
    n_args = n_params + len(out_names)
    sharded = jax.jit(shard_map(
        _body, mesh=mesh,
        in_specs=(PartitionSpec("core"),) * n_args,
        out_specs=(PartitionSpec("core"),) * len(out_names),
        check_rep=False))
    _EXEC_CACHE = (sharded, mesh, in_names, out_names, out_shapes)
    return _EXEC_CACHE


def _run_spmd(in_maps):
    """Execute on all cores; returns list of per-core {name: np.ndarray}."""
    global LAST_DEV_ARGS, LAST_OUT_NAMES
    sharded, mesh, in_names, out_names, out_shapes = _get_executor()
    sh = NamedSharding(mesh, PartitionSpec("core"))
    args = [np.concatenate([im[name] for im in in_maps], axis=0)
            for name in in_names]
    for shape, dtype in out_shapes:
        args.append(np.zeros((NCORES * shape[0],) + shape[1:], dtype))
    dev_args = [jax.device_put(a, sh) for a in args]
    LAST_DEV_ARGS = dev_args
    LAST_OUT_NAMES = out_names
    outs = sharded(*dev_args)
    jax.block_until_ready(outs)
    results = []
    for c in range(NCORES):
        res = {}
        for i, name in enumerate(out_names):
            g = np.asarray(outs[i])
            d0 = g.shape[0] // NCORES
            res[name] = g[c * d0:(c + 1) * d0]
        results.append(res)
    return results


def _layout_w(w, c):
    """[H, D, P] global weights -> per-core [128, D] stationary layout:
    out[di, dc*128 + (h*64+p)] = w[2c+h, dc*128+di, p]"""
    wl = np.transpose(w[HL * c:HL * (c + 1)], (1, 0, 2)).reshape(D, P2)
    wl = wl.reshape(DCH, 128, P2).transpose(1, 0, 2).reshape(128, DCH * P2)
    return np.ascontiguousarray(wl, dtype=np.float32)


def make_in_maps(x, Qw, Kw, Vw, W_fin):
    import ml_dtypes
    x = np.asarray(x, dtype=np.float32)
    Qw = np.asarray(Qw, dtype=np.float32)
    Kw = np.asarray(Kw, dtype=np.float32)
    Vw = np.asarray(Vw, dtype=np.float32)
    W_fin = np.asarray(W_fin, dtype=np.float32)

    # u-major column permutation: position i = u*128 + r  <->  s = 16*r + u
    xt = np.ascontiguousarray(x.T)                      # [D, S]
    xtp = np.ascontiguousarray(
        xt.reshape(D, 128, 16).transpose(0, 2, 1).reshape(D, S))
    ident = np.eye(128, dtype=np.float32)
    # wf: natural W_fin contraction chunks [128, c*F + f], bf16
    wf = np.ascontiguousarray(
        W_fin.reshape(NC_F, 128, F).transpose(1, 0, 2).reshape(128, NC_F * F)
    ).astype(ml_dtypes.bfloat16)

    in_maps = []
    for c in range(NCORES):
        in_maps.append({
            "xt": xtp,
            "wq": _layout_w(Qw, c),
            "wk": _layout_w(Kw, c),
            "wv": _layout_w(Vw, c),
            "wf": wf,
            "ident": ident,
        })
    return in_maps


def assemble_out(results, b_fin):
    b_fin = np.asarray(b_fin, dtype=np.float32)
    cores = []
    for c in range(NCORES):
        buf = results[c]["out"].astype(np.float32)      # [128, 2048]
        v = buf.reshape(2, 64, 2, 2, 512)               # [h, r', q, fc, f]
        cores.append(v.transpose(0, 2, 1, 3, 4).reshape(256, F))
    out = np.concatenate(cores, axis=0)
    return (out + b_fin).astype(np.float32)


def kernel(x, Qw, Kw, Vw, W_fin, b_fin):
    in_maps = make_in_maps(x, Qw, Kw, Vw, W_fin)
    results = _run_spmd(in_maps)
    return assemble_out(results, b_fin)


# revision 28
# speedup vs baseline: 1.0848x; 1.0274x over previous
"""Trainium2 Bass kernel: sigmoid multi-head attention (16 heads, S=2048, D=1024,
P=64) + final linear, head-sharded across 8 NeuronCores (2 heads/core).

Reference semantics: concat = attn.reshape(S, -1) is a RAW reshape of the
contiguous [H, S, P] attn array, so output row i draws only from head
h = i // 128:  out[h*128 + r, f] = sum_{u,p} attn[h, 16r+u, p] * W_fin[u*64+p, f]
with u = s%16, r = s//16.  Core c (heads 2c, 2c+1) owns output rows
[256c, 256c+256); the host gather is a concatenation.

v2 design (cost-model driven):
  * The whole kernel runs in a PERMUTED s/t order: position i = u*128 + r
    (u-major), applied on the host to x^T's columns.  Projections, scores,
    sigmoid and attention all inherit the permutation (attention contracts
    over all t, any consistent order is exact).  Benefit: a 128-wide
    permuted-s tile = one u value for all 128 r, so the attention output is
    already grouped by the final linear's contraction chunks.
  * Scores: scoreT [t,128, (h, s,512)] psum per (block, t-tile), fp32r,
    2 matmuls of N=512.  Sigmoid on ScalarE -> sc bf16 SBUF (ScalarE is the
    pipeline bottleneck: 64 x ~1.04us).
  * Attention in NATURAL orientation: out[s,128, p,64] per (tile, head),
    lhsT = sc slice (bf16), rhs = V-natural column half (bf16), N=64 ->
    half the PE cost of the attnT orientation.  All 8 accumulation regions
    of a block share ONE psum bank, pre-seeded by a K=1 zero matmul with
    start=True (sets the whole bank's has_written bits); real matmuls use
    start=False so concurrent per-region accumulation is safe.
  * Final linear: concatT chunks fT_c [k=(u2,p),128, (q:(h,r'))256] bf16 are
    built from the attention output by DMA transpose + 4 small remap DMAs per
    tile, then out[(h,r'),128, f,512] = sum_c fT_c^T @ W_fin[c-chunk] with
    W_fin natural bf16 as the moving operand: 32 matmuls of N=512 total
    (4x fewer PE cycles than the per-u K=64 scheme).  Accumulated in 2
    psum passes (after block 1 and block 3) with DVE adds into SBUF.
  * Emission is software-pipelined: sig(i) | scores(i+2) | attn(i-2) plus
    "slack" projection/final pieces, so stage-P and the final hide under the
    ScalarE-bound sigmoid stream.
"""

import os

os.environ.setdefault("BASS_NEVER_TRACE", "1")

import numpy as np
from contextlib import ExitStack

import jax
import concourse.bacc as bacc
import concourse.bass as bass
import concourse.mybir as mybir
import concourse.tile as tile
from concourse.bass2jax import (
    _bass_exec_p,
    install_neuronx_cc_hook,
    partition_id_tensor,
)
from jax.experimental.shard_map import shard_map
from jax.sharding import Mesh, NamedSharding, PartitionSpec

S, D, H, P, F = 2048, 1024, 16, 64, 1024
NCORES = 8
HL = H // NCORES          # heads per core = 2
P2 = HL * P               # stacked head dim = 128
DCH = D // 128            # 8 contraction chunks
NB = S // 512             # 4 s-blocks (permuted order)
NT = S // 128             # 16 t-tiles
NC_F = 8                  # final-linear contraction chunks of 128

FP32 = mybir.dt.float32
FP32R = mybir.dt.float32r
BF16 = mybir.dt.bfloat16
SIGMOID = mybir.ActivationFunctionType.Sigmoid
ADD = mybir.AluOpType.add


def build_kernel(ctx: ExitStack, tc: tile.TileContext, xt_d, wq_d, wk_d, wv_d,
                 wf_d, id_d, out_d):
    nc = tc.nc

    const_pool = ctx.enter_context(tc.tile_pool(name="const", bufs=1))
    w_pool = ctx.enter_context(tc.tile_pool(name="wts", bufs=1))
    qk_pool = ctx.enter_context(tc.tile_pool(name="qk", bufs=1))
    xt_pool = ctx.enter_context(tc.tile_pool(name="xt", bufs=4))
    sc_pool = ctx.enter_context(tc.tile_pool(name="sc", bufs=6))
    ab_pool = ctx.enter_context(tc.tile_pool(name="ab", bufs=2))
    abt_pool = ctx.enter_context(tc.tile_pool(name="abt", bufs=1))
    ft_pool = ctx.enter_context(tc.tile_pool(name="ft", bufs=1))
    fa_pool = ctx.enter_context(tc.tile_pool(name="fa", bufs=1))
    ot_pool = ctx.enter_context(tc.tile_pool(name="ot", bufs=1))

    # ---- PSUM: pp (P-stage, 2 banks) | ps_s (4 banks) | pa (2 banks) ----
    pp_pool = tc.alloc_tile_pool(name="pp", bufs=1, space="PSUM", side="right")
    ps_s_pool = ctx.enter_context(tc.tile_pool(name="ps_s", bufs=2, space="PSUM"))
    pa_pool = ctx.enter_context(tc.tile_pool(name="pa", bufs=2, space="PSUM"))

    ident = const_pool.tile([128, 128], FP32, tag="ident")
    nc.gpsimd.dma_start(ident, id_d)
    identb = const_pool.tile([128, 128], BF16, tag="identb")
    nc.gpsimd.dma_start(identb, id_d)      # gpsimd DMA casts fp32 -> bf16
    zl = const_pool.tile([1, 128], BF16, tag="zl")
    zr = const_pool.tile([1, 512], BF16, tag="zr")
    nc.vector.memset(zl, 0.0)
    nc.vector.memset(zr, 0.0)

    # ---------------- input DMAs (three lanes: sync / scalar / gpsimd) ------
    # gpsimd queue: ident, wk, wq, wv, then x shares (weights gate stage-P)
    wk = w_pool.tile([128, D], FP32R, tag="wk")
    nc.gpsimd.dma_start(wk, wk_d.bitcast(FP32R))
    wq = w_pool.tile([128, D], FP32R, tag="wq")
    nc.gpsimd.dma_start(wq, wq_d.bitcast(FP32R))
    wv = w_pool.tile([128, D], FP32R, tag="wv")
    nc.gpsimd.dma_start(wv, wv_d.bitcast(FP32R))

    xts = [[None] * DCH for _ in range(NB)]

    def emit_x(sb, ds, engine):
        for d in ds:
            t = xt_pool.tile([128, 512], FP32R, tag=f"x{d}", name=f"xt{sb}_{d}")
            engine.dma_start(
                t, xt_d[d * 128:(d + 1) * 128,
                        sb * 512:(sb + 1) * 512].bitcast(FP32R))
            xts[sb][d] = t

    # x0/x1 land first (critical path to the first sigmoids); the ScalarE
    # lane must finish all its DMAs before the first sigmoid is enqueued.
    emit_x(0, range(0, 4), nc.sync)
    emit_x(0, range(4, 8), nc.scalar)
    # preload the sigmoid act table while ScalarE is otherwise idle
    warmt = const_pool.tile([1, 128], BF16, tag="warmt")
    nc.scalar.activation(warmt, zl, SIGMOID)
    emit_x(1, range(0, 4), nc.sync)
    emit_x(1, range(4, 8), nc.gpsimd)
    emit_x(2, range(0, 8), nc.sync)
    emit_x(3, range(0, 8), nc.gpsimd)

    wf = w_pool.tile([128, NC_F * F], BF16, tag="wf")
    for c in range(NC_F):
        nc.sync.dma_start(wf[:, c * F:(c + 1) * F],
                          wf_d[:, c * F:(c + 1) * F])

    qt2 = qk_pool.tile([128, S], FP32R, tag="qt2")   # [p2, s-perm]
    kt2 = qk_pool.tile([128, S], FP32R, tag="kt2")   # [p2, t-perm]
    vt2 = qk_pool.tile([128, S], FP32, tag="vt2")    # [p2, t-perm]
    v2n = qk_pool.tile([128, S], BF16, tag="v2n")    # [t_in, j*128 + p2]

    # ---------------- stage-P pieces (emission closures) ----------------
    pj_live = {}

    def piece_proj(w, dst, sb, wname, half=None):
        """Projection piece: half 0 = chunks 0-3 (allocates the pj slot),
        half 1 = chunks 4-7 + psum->SBUF copy, None = whole projection.
        The two halves of one projection must be emitted with no other
        projection's pieces in between (single pj slot)."""
        cols = slice(sb * 512, (sb + 1) * 512)

        def f():
            if half in (0, None):
                pj_live["pj"] = pp_pool.tile([128, 512], FP32, tag="pj",
                                             name=f"pj_{wname}{sb}")
            pj = pj_live["pj"]
            ds = {0: range(0, 4), 1: range(4, 8), None: range(8)}[half]
            for d in ds:
                nc.tensor.matmul(pj, w[:, d * 128:(d + 1) * 128], xts[sb][d],
                                 start=(d == 0), stop=(d == DCH - 1))
            if half in (1, None):
                nc.vector.tensor_copy(dst[:, cols], pj)
        return f

    def piece_vtr(j0, j1):
        """Transpose 128-col tiles [j0, j1) of vt2 into v2n (bf16)."""
        def f():
            for j in range(j0, j1):
                pt = pp_pool.tile([128, 128], FP32, tag="pt", name=f"pt{j}")
                nc.tensor.transpose(pt, vt2[:, j * 128:(j + 1) * 128], ident)
                nc.vector.tensor_copy(v2n[:, j * 128:(j + 1) * 128], pt)
        return f

    # ---------------- stream state ----------------
    pa_tiles = [None] * NB
    sc_tiles = [None] * (NB * NT)
    ps_tiles = [None] * (NB * NT)
    ft_tiles = [None] * NC_F
    fa_tiles = {}
    psf_holder = {}

    # stream order: iters 0..31 interleave blocks 0/1 (b = i%2, j = i//2),
    # then block 2 (j = i-32), then block 3 (j = i-48).  This spreads the
    # t-tile (K/V) deadlines over 32 iters and leaves only block 3's
    # restack + final chunk in the tail.
    def BJ(i):
        if i < 32:
            return i % 2, i // 2
        if i < 48:
            return 2, i - 32
        return 3, i - 48

    def emit_zero(b):
        pa = pa_pool.tile([128, 512], FP32, tag="pa", name=f"pa{b}")
        nc.tensor.matmul(pa, zl, zr, start=True, stop=False,
                         skip_group_check=True)
        pa_tiles[b] = pa

    def emit_scores(i):
        b, j = BJ(i)
        ps = ps_s_pool.tile([128, 1024], FP32, tag="ps_s", name=f"ps{b}_{j}")
        t0, s0 = j * 128, b * 512
        nc.tensor.matmul(ps[:, 0:512], kt2[0:64, t0:t0 + 128],
                         qt2[0:64, s0:s0 + 512])
        nc.tensor.matmul(ps[:, 512:1024], kt2[64:128, t0:t0 + 128],
                         qt2[64:128, s0:s0 + 512])
        ps_tiles[b * NT + j] = ps

    def emit_sig(i):
        b, j = BJ(i)
        sc = sc_pool.tile([128, 1024], BF16, tag="sc", name=f"sc{b}_{j}")
        nc.scalar.activation(sc, ps_tiles[b * NT + j], SIGMOID, scale=1.0 / P)
        sc_tiles[b * NT + j] = sc

    def emit_attn(i):
        b, j = BJ(i)
        pa = pa_tiles[b]
        sc = sc_tiles[b * NT + j]
        t0 = j * 128
        for t4 in range(4):
            for h in range(2):
                nc.tensor.matmul(
                    pa[:, t4 * 128 + h * 64: t4 * 128 + h * 64 + 64],
                    sc[:, h * 512 + t4 * 128: h * 512 + (t4 + 1) * 128],
                    v2n[:, t0 + h * 64: t0 + h * 64 + 64],
                    start=False, stop=(j == NT - 1), skip_group_check=True)

    def emit_abchain(b, tail=False):
        """After attn(b,15): stage, transpose and remap the block's 4 tiles.
        ft layout: [k=(u2,p), (q, h, r')]; each (tau, h) remap is one DMA with
        a 3D dest AP.  In the tail, the idle ScalarE does the staging copy and
        all three DMA-capable engines share the transposes/remaps."""
        ab = ab_pool.tile([128, 512], BF16, tag="ab", name=f"ab{b}")
        if tail:
            nc.scalar.activation(ab, pa_tiles[b],
                                 mybir.ActivationFunctionType.Copy)
        else:
            nc.vector.tensor_copy(ab, pa_tiles[b])
        lanes = ((nc.scalar, nc.sync) if tail
                 else (nc.gpsimd, nc.sync))
        li = 0
        for t4 in range(4):
            tau = 4 * b + t4
            c, u2 = tau // 2, tau % 2
            abt = abt_pool.tile([128, 128], BF16, tag=f"abt{tau}",
                                name=f"abt{tau}")
            nc.sync.dma_start_transpose(abt, ab[:, t4 * 128:(t4 + 1) * 128])
            if u2 == 0:
                ft_tiles[c] = ft_pool.tile([128, 256], BF16, tag=f"ft{c}",
                                           name=f"ft{c}")
            ft = ft_tiles[c]
            for h in range(2):
                eng = lanes[li % len(lanes)]
                li += 1
                # ft cols are (q, h, r'); abt cols are r = 64q + r'.
                # rearrange + plain slice only — int-indexed APs break
                # Tile's write-dependency tracking (observed race).
                dst = ft[u2 * 64:(u2 + 1) * 64, :].rearrange(
                    "p (q x) -> p q x", q=2, x=128)[:, :, h * 64:(h + 1) * 64]
                sr = abt[h * 64:(h + 1) * 64, :].rearrange(
                    "p (q r) -> p q r", q=2, r=64)
                eng.dma_start(dst, sr)

    def final_pieces(phase):
        """phase 0: chunks 0-3 -> fa copy; 1: chunks 4-5 -> fa add;
        2: chunks 6-7 -> ot = fa + psf, DMA out."""
        pieces = []
        state = {}
        cs = ((0, 1, 2, 3), (4, 5), (6, 7))[phase]

        def mk_mm(q, fc, sub):
            def f():
                psf_pool = psf_holder["pool"]
                if sub[0] == cs[0]:
                    state[(q, fc)] = psf_pool.tile(
                        [128, 512], FP32, tag="psf", name=f"psf{phase}{q}{fc}")
                psf = state[(q, fc)]
                for c in sub:
                    lhsT = ft_tiles[c][:, q * 128:(q + 1) * 128]
                    nc.tensor.matmul(
                        psf, lhsT,
                        wf[:, c * F + fc * 512: c * F + fc * 512 + 512],
                        start=(c == cs[0]), stop=(c == cs[-1]))
            return f

        def mk_tail(q, fc):
            def f():
                psf = state[(q, fc)]
                if phase == 0:
                    fa = fa_pool.tile([128, 512], FP32, tag=f"fa{q}{fc}",
                                      name=f"fa{q}{fc}")
                    nc.vector.tensor_copy(fa, psf)
                    fa_tiles[(q, fc)] = fa
                elif phase == 1:
                    nc.vector.tensor_tensor(fa_tiles[(q, fc)],
                                            fa_tiles[(q, fc)], psf, ADD)
                else:
                    ot = ot_pool.tile([128, 512], BF16, tag=f"ot{q}{fc}",
                                      name=f"ot{q}{fc}")
                    nc.vector.tensor_tensor(ot, fa_tiles[(q, fc)], psf, ADD)
                    eng = nc.sync if (q + fc) % 2 == 0 else nc.scalar
                    eng.dma_start(
                        out_d[:, (q * 2 + fc) * 512:(q * 2 + fc + 1) * 512],
                        ot)
            return f

        for q in range(2):
            for fc in range(2):
                if phase == 0:
                    pieces.append(mk_mm(q, fc, (0, 1)))
                    pieces.append(mk_mm(q, fc, (2, 3)))
                else:
                    pieces.append(mk_mm(q, fc, cs))
                pieces.append(mk_tail(q, fc))
        return pieces

    # ---------------- prologue ----------------
    # Emission order IS dependency order: never emit a consumer before its
    # producer (the scheduler honors emission order per engine).
    # K0 and Q0 run d-interleaved into one (otherwise idle) ps_s tile —
    # its two banks hold two independent accumulation groups — so both
    # finish right after the last x(sb0) tile lands.
    pkq = ps_s_pool.tile([128, 1024], FP32, tag="ps_s", name="pkq0")
    for d in range(DCH):
        nc.tensor.matmul(pkq[:, 0:512], wk[:, d * 128:(d + 1) * 128],
                         xts[0][d], start=(d == 0), stop=(d == DCH - 1))
        nc.tensor.matmul(pkq[:, 512:1024], wq[:, d * 128:(d + 1) * 128],
                         xts[0][d], start=(d == 0), stop=(d == DCH - 1))
    nc.vector.tensor_copy(kt2[:, 0:512], pkq[:, 0:512])
    nc.vector.tensor_copy(qt2[:, 0:512], pkq[:, 512:1024])
    emit_zero(0)
    emit_zero(1)
    emit_scores(0)
    # Q(sb1) via the other ps_s slot (frees up before scores(2) needs it)
    pq1 = ps_s_pool.tile([128, 1024], FP32, tag="ps_s", name="pq1")
    for d in range(DCH):
        nc.tensor.matmul(pq1[:, 0:512], wq[:, d * 128:(d + 1) * 128],
                         xts[1][d], start=(d == 0), stop=(d == DCH - 1))
    nc.vector.tensor_copy(qt2[:, 512:1024], pq1[:, 0:512])
    emit_scores(1)
    piece_proj(wv, vt2, 0, "v")()
    piece_vtr(0, 2)()
    piece_vtr(2, 4)()

    # ---------------- slack schedule ----------------
    sched = {i: [] for i in range(NB * NT + 8)}

    def put_proj(i0, w, dst, sb, wname):
        sched[i0].append(piece_proj(w, dst, sb, wname, 0))
        sched[i0 + 1].append(piece_proj(w, dst, sb, wname, 1))

    put_proj(0, wk, kt2, 1, "k")
    put_proj(2, wv, vt2, 1, "v")
    sched[4].append(piece_vtr(4, 6))
    sched[5].append(piece_vtr(6, 8))
    put_proj(8, wk, kt2, 2, "k")
    put_proj(10, wv, vt2, 2, "v")
    sched[12].append(piece_vtr(8, 10))
    sched[13].append(piece_vtr(10, 12))
    put_proj(16, wk, kt2, 3, "k")
    put_proj(18, wv, vt2, 3, "v")
    sched[20].append(piece_vtr(12, 14))
    sched[21].append(piece_vtr(14, 16))
    put_proj(26, wq, qt2, 2, "q")
    put_proj(30, wq, qt2, 3, "q")
    sched[33].append(lambda: emit_zero(2))
    sched[48].append(lambda: emit_zero(3))

    # ---------------- main stream ----------------
    for i in range(NB * NT):
        emit_sig(i)
        if i + 2 < NB * NT:
            emit_scores(i + 2)
        if i - 2 >= 0:
            emit_attn(i - 2)
            bb, jj = BJ(i - 2)
            if jj == NT - 1:
                emit_abchain(bb)
                if bb == 1:
                    # all stage-P work is emitted; free its PSUM for psf
                    pp_pool.release()
                    cm = tc.tile_pool(name="psf", bufs=2, space="PSUM")
                    psf_holder["cm"] = cm
                    psf_holder["pool"] = cm.__enter__()
                    for k, p in enumerate(final_pieces(0)):
                        sched[36 + (k * 10) // 12].append(p)
                elif bb == 2:
                    for k, p in enumerate(final_pieces(1)):
                        sched[52 + (k * 8) // 8].append(p)
        for p in sched[i]:
            p()

    # ---------------- tail ----------------
    emit_attn(NB * NT - 2)
    emit_attn(NB * NT - 1)
    emit_abchain(NB - 1, tail=True)
    for p in final_pieces(2):
        p()
    psf_holder["cm"].__exit__(None, None, None)


def build_bass(replicas: int = 1) -> bass.Bass:
    nc = bacc.Bacc("TRN2", target_bir_lowering=False, debug=False,
                   num_devices=NCORES)
    xt_d = nc.dram_tensor("xt", [D, S], FP32, kind="ExternalInput").ap()
    wq_d = nc.dram_tensor("wq", [128, D], FP32, kind="ExternalInput").ap()
    wk_d = nc.dram_tensor("wk", [128, D], FP32, kind="ExternalInput").ap()
    wv_d = nc.dram_tensor("wv", [128, D], FP32, kind="ExternalInput").ap()
    wf_d = nc.dram_tensor("wf", [128, NC_F * F], BF16, kind="ExternalInput").ap()
    id_d = nc.dram_tensor("ident", [128, 128], FP32, kind="ExternalInput").ap()
    # compact layout: [ (h,r'), (q,fc)*512 + f' ] — host unscatters rows
    out_d = nc.dram_tensor("out", [128, 4 * 512], BF16, kind="ExternalOutput").ap()
    with tile.TileContext(nc) as tc:
        for _ in range(replicas):
            with ExitStack() as ctx:
                build_kernel(ctx, tc, xt_d, wq_d, wk_d, wv_d, wf_d, id_d,
                             out_d)
    nc.finalize()
    return nc


_NC_CACHE = None
_EXEC_CACHE = None
LAST_DEV_ARGS = None
LAST_OUT_NAMES = None


def _get_nc():
    global _NC_CACHE
    if _NC_CACHE is None:
        _NC_CACHE = build_bass()
    return _NC_CACHE


def _get_executor():
    """Compile the SPMD PJRT executable once (mirrors bass2jax.run_bass_via_pjrt,
    minus output-buffer donation)."""
    global _EXEC_CACHE
    if _EXEC_CACHE is not None:
        return _EXEC_CACHE
    import concourse.mybir as mybir

    nc = _get_nc()
    install_neuronx_cc_hook()
    partition_name = (nc.partition_id_tensor.name
                      if nc.partition_id_tensor else None)
    in_names, out_names, out_avals = [], [], []
    out_shapes = []
    for alloc in nc.m.functions[0].allocations:
        if not isinstance(alloc, mybir.MemoryLocationSet):
            continue
        name = alloc.memorylocations[0].name
        if alloc.kind == "ExternalInput":
            if name != partition_name:
                in_names.append(name)
        elif alloc.kind == "ExternalOutput":
            shape = tuple(alloc.tensor_shape)
            dtype = mybir.dt.np(alloc.dtype)
            out_names.append(name)
            out_avals.append(jax.core.ShapedArray(shape, dtype))
            out_shapes.append((shape, dtype))
    n_params = len(in_names)
    all_names = list(in_names) + list(out_names)
    if partition_name is not None:
        all_names.append(partition_name)

    def _body(*args):
        operands = list(args)
        if partition_name is not None:
            operands.append(partition_id_tensor())
        outs = _bass_exec_p.bind(
            *operands,
            out_avals=tuple(out_avals),
            in_names=tuple(all_names),
            out_names=tuple(out_names),
            lowering_input_output_aliases=(),
            sim_require_finite=True,
            sim_require_nnan=True,
            nc=nc,
        )
        return tuple(outs)

    devices = jax.devices()[:NCORES]
    mesh = Mesh(np.asarray(devices), ("core",))
    n_args = n_params + len(out_names)
    sharded = jax.jit(shard_map(
        _body, mesh=mesh,
        in_specs=(PartitionSpec("core"),) * n_args,
        out_specs=(PartitionSpec("core"),) * len(out_names),
        check_rep=False))
    _EXEC_CACHE = (sharded, mesh, in_names, out_names, out_shapes)
    return _EXEC_CACHE


def _run_spmd(in_maps):
    """Execute on all cores; returns list of per-core {name: np.ndarray}."""
    global LAST_DEV_ARGS, LAST_OUT_NAMES
    sharded, mesh, in_names, out_names, out_shapes = _get_executor()
    sh = NamedSharding(mesh, PartitionSpec("core"))
    args = [np.concatenate([im[name] for im in in_maps], axis=0)
            for name in in_names]
    for shape, dtype in out_shapes:
        args.append(np.zeros((NCORES * shape[0],) + shape[1:], dtype))
    dev_args = [jax.device_put(a, sh) for a in args]
    LAST_DEV_ARGS = dev_args
    LAST_OUT_NAMES = out_names
    outs = sharded(*dev_args)
    jax.block_until_ready(outs)
    results = []
    for c in range(NCORES):
        res = {}
        for i, name in enumerate(out_names):
            g = np.asarray(outs[i])
            d0 = g.shape[0] // NCORES
            res[name] = g[c * d0:(c + 1) * d0]
        results.append(res)
    return results


def _layout_w(w, c):
    """[H, D, P] global weights -> per-core [128, D] stationary layout:
    out[di, dc*128 + (h*64+p)] = w[2c+h, dc*128+di, p]"""
    wl = np.transpose(w[HL * c:HL * (c + 1)], (1, 0, 2)).reshape(D, P2)
    wl = wl.reshape(DCH, 128, P2).transpose(1, 0, 2).reshape(128, DCH * P2)
    return np.ascontiguousarray(wl, dtype=np.float32)


def make_in_maps(x, Qw, Kw, Vw, W_fin):
    import ml_dtypes
    x = np.asarray(x, dtype=np.float32)
    Qw = np.asarray(Qw, dtype=np.float32)
    Kw = np.asarray(Kw, dtype=np.float32)
    Vw = np.asarray(Vw, dtype=np.float32)
    W_fin = np.asarray(W_fin, dtype=np.float32)

    # u-major column permutation: position i = u*128 + r  <->  s = 16*r + u
    xt = np.ascontiguousarray(x.T)                      # [D, S]
    xtp = np.ascontiguousarray(
        xt.reshape(D, 128, 16).transpose(0, 2, 1).reshape(D, S))
    ident = np.eye(128, dtype=np.float32)
    # wf: natural W_fin contraction chunks [128, c*F + f], bf16
    wf = np.ascontiguousarray(
        W_fin.reshape(NC_F, 128, F).transpose(1, 0, 2).reshape(128, NC_F * F)
    ).astype(ml_dtypes.bfloat16)

    in_maps = []
    for c in range(NCORES):
        in_maps.append({
            "xt": xtp,
            "wq": _layout_w(Qw, c),
            "wk": _layout_w(Kw, c),
            "wv": _layout_w(Vw, c),
            "wf": wf,
            "ident": ident,
        })
    return in_maps


def assemble_out(results, b_fin):
    b_fin = np.asarray(b_fin, dtype=np.float32)
    cores = []
    for c in range(NCORES):
        buf = results[c]["out"].astype(np.float32)      # [128, 2048]
        v = buf.reshape(2, 64, 2, 2, 512)               # [h, r', q, fc, f]
        cores.append(v.transpose(0, 2, 1, 3, 4).reshape(256, F))
    out = np.concatenate(cores, axis=0)
    return (out + b_fin).astype(np.float32)


def kernel(x, Qw, Kw, Vw, W_fin, b_fin):
    in_maps = make_in_maps(x, Qw, Kw, Vw, W_fin)
    results = _run_spmd(in_maps)
    return assemble_out(results, b_fin)
